# revision 11
# baseline (speedup 1.0000x reference)
"""Trainium2 Bass kernel for BlockDecomposedSSMAttention.

Math: y[b,s,:] = x[b,s,:] @ B.T @ A @ C.T   (no cross-block recurrence)
 ==>  y = x @ W  with  W = B.T @ A @ C.T

Distribution over the 8 NeuronCores (grid = 2 row-groups x 4 col-quarters):
  core c = (rg, cq):  computes y[rg*8192:(rg+1)*8192, cq*256:(cq+1)*256]
  - x rows are split 2 ways (8192 rows/core, read by 4 cores each).
  - Each core only needs W[:, cq*256:(cq+1)*256], so the W-build stages
    shrink 4x vs full-W-per-core:  T = A @ Ct_q   (1024x256)
                                   W_q = B.T @ T  (1024x256)

Timing model (measured: preamble ~7us, first DMA byte ~8.4us, per-core
input stream capped ~350 GB/s with no cross-queue aggregation,
216ns/512-row matmul; collectives are 5-10x degraded here, so the
duplicated per-core build beats any cross-core W sharing):
  - params are HOST-PACKED into one tensor in exact consumption order
    (per kt: ct 256 cols | at 1024 cols, then bt), so one small first
    piece (0.31 MB, 2.5-KB lines) starts the PE at ~10us and the build
    runs PE-bound to ~24us with zero issue-order stalls.
  - x is chunk-major ([P, chunk, kt, m] -> 8-KB DMA lines) and streams
    JIT behind params under the 55.4us main loop.
  - main loop: W stationary, x moving at N=512, 4 psum banks per
    m-group; y.T out via gpsimd SWDGE (off the input queue). y_out is
    [P, ot, m] so the final 256-row group drains with ONE sync-HWDGE
    DMA right after its two (vector+scalar, parallel) psum copies.
  - All matmul operands bf16 (1 cyc/row), PSUM fp32; y written bf16,
    host transposes + upcasts. Host does layout marshalling only.
"""

import os
import sys

import numpy as np

if "/opt/trn_rl_repo" not in sys.path:
    sys.path.insert(0, "/opt/trn_rl_repo")

import ml_dtypes

BF16 = ml_dtypes.bfloat16

BATCH, SEQ, D = 4, 4096, 1024
NCORES = 8
RG, CQ = 2, 4                 # row-groups x col-quarters
ROWS = BATCH * SEQ            # 16384
MSH = ROWS // RG              # 8192 rows per core
OD = D // CQ                  # 256 output cols per core
P = 128
KT = D // P                   # 8 contraction tiles
MC = 512                      # moving chunk of m in the main loop
NMC = MSH // MC               # 16 m-chunks
NOT = OD // P                 # 2 o'-tiles

KTW = OD + D                  # 1280 packed param cols per kt (ct | at)
BTO = KT * KTW                # 10240: offset of the bt section
PW = BTO + KT * D             # 18432 packed param cols total

_CACHE: dict = {}


def _build_nc():
    import concourse.mybir as mybir
    import concourse.tile as tile
    from concourse import bacc

    f32 = mybir.dt.float32
    bf16 = mybir.dt.bfloat16

    nc = bacc.Bacc(
        "TRN2", target_bir_lowering=False, debug=False, num_devices=NCORES,
        num_swdge_queues=1,
    )

    # Per-core inputs (bf16, contraction dim on partitions):
    #   pk [kp, kt*1280 + o]        = C[cq*256+o, kt*128+kp]   (ct slice)
    #   pk [kp, kt*1280 + 256 + j]  = A[j, kt*128+kp]          (A.T)
    #   pk [jp, 10240 + jt*1024 + i] = B[jt*128+jp, i]         (B)
    #   xt [ip, c, io, m] = x2[rg*8192 + c*512 + m, io*128+ip] (x shard)
    pk_in = nc.dram_tensor("pk_in", [P, PW], bf16, kind="ExternalInput")
    xt = nc.dram_tensor("xt", [P, NMC, KT, MC], bf16, kind="ExternalInput")
    # y.T shard as [p, o', m]; host transposes + upcasts.
    y_out = nc.dram_tensor("y_out", [P, NOT, MSH], bf16, kind="ExternalOutput")

    def ct_ap(sb, kt):
        return sb[:, kt * KTW : kt * KTW + OD]

    def at_ap(sb, kt, jt):
        o = kt * KTW + OD + jt * P
        return sb[:, o : o + P]

    def bt_ap(sb, jt, it):
        o = BTO + jt * D + it * P
        return sb[:, o : o + P]

    with tile.TileContext(nc) as tc:
        with (
            tc.tile_pool(name="big", bufs=1) as big,
            tc.tile_pool(name="ycopy", bufs=8) as ycopy,
            tc.tile_pool(name="psp", bufs=8, space="PSUM") as psp,
        ):
            pk_sb = big.tile([P, PW], bf16)
            t_sb = big.tile([P, KT, OD], bf16)
            w_sb = big.tile([P, KT, OD], bf16)
            xt_sb = big.tile([P, NMC, KT, MC], bf16)

            # ---- input DMAs, all on the single Sync HWDGE queue in exact
            # consumption order (FIFO; per-core bandwidth doesn't aggregate
            # across queues, so one well-ordered stream is optimal). y goes
            # out via the otherwise-idle Scalar HWDGE queue: off the input
            # FIFO, and avoids the 3.4us SWDGE drain in the postamble.
            def pget(lo, hi):
                nc.sync.dma_start(pk_sb[:, lo:hi], pk_in.ap()[:, lo:hi])

            # per-kt pieces: semaphore granularity matches stage-1's
            # 0.87us/kt consumption, so pacing stalls stay sub-100ns.
            # kt0 is split after at-jt3: the first 4 matmuls bridge the
            # second piece's ~0.9us completion-semaphore latency.
            pget(0, OD + 4 * P)
            pget(OD + 4 * P, KTW)
            for kt in range(1, KT):
                pget(kt * KTW, (kt + 1) * KTW)
            # per-jt bt pieces: stage-2's first matmuls no longer wait on a
            # 1-MiB block; consumption granularity 0.87us/jt matches.
            for jt in range(KT):
                pget(BTO + jt * D, BTO + (jt + 1) * D)
            # x stream: two 1-chunk pieces (first m-group), then 2 MiB pairs
            nc.sync.dma_start(xt_sb[:, 0:1], xt.ap()[:, 0:1])
            for c in range(7):
                nc.sync.dma_start(
                    xt_sb[:, 2 * c + 1 : 2 * c + 3],
                    xt.ap()[:, 2 * c + 1 : 2 * c + 3],
                )
            nc.sync.dma_start(xt_sb[:, 15:16], xt.ap()[:, 15:16])

            # ---- stage 1: T = A @ Ct_q  [1024 x 256], kt-outer over a
            # single 8-bank pass (j-tiles 0..7), paced by the packed stream.
            ps1 = [psp.tile([P, MC], f32, name="psp") for j in range(KT)]
            for kt in range(KT):
                for jt in range(KT):
                    nc.tensor.matmul(
                        ps1[jt][:, 0:OD],
                        at_ap(pk_sb, kt, jt),
                        ct_ap(pk_sb, kt),
                        start=(kt == 0),
                        stop=(kt == KT - 1),
                    )
            for jt in range(KT):
                if jt < 4:
                    nc.vector.tensor_copy(t_sb[:, jt, :], ps1[jt][:, 0:OD])
                else:
                    nc.scalar.copy(t_sb[:, jt, :], ps1[jt][:, 0:OD])

            # ---- stage 2: W_q = B.T @ T  [1024 x 256], jt-outer over a
            # single 8-bank pass (it-tiles 0..7), paced by the bt stream.
            ps2 = [psp.tile([P, MC], f32, name="psp") for i in range(KT)]
            for jt in range(KT):
                for it in range(KT):
                    nc.tensor.matmul(
                        ps2[it][:, 0:OD],
                        bt_ap(pk_sb, jt, it),
                        t_sb[:, jt, :],
                        start=(jt == 0),
                        stop=(jt == KT - 1),
                    )
            # vector (fast) copies it 0-3, scalar (slower ACTIVATE) 4-7;
            # the main loop consumes it-tiles in copy-completion order.
            for it in range(KT):
                if it < 4:
                    nc.vector.tensor_copy(w_sb[:, it, :], ps2[it][:, 0:OD])
                else:
                    nc.scalar.copy(w_sb[:, it, :], ps2[it][:, 0:OD])

            # ---- main: y_q.T = W_q.T @ x.T  [256 x 8192] ----
            # W stationary (reused across m), x moving at N=512.
            # groups: pairs of 512-chunks (= one 2 MiB x DMA piece) -> 4 psum
            # banks per group; final group is a single 256-row chunk drained
            # by ONE dma on the (by then idle) sync HWDGE.
            groups = [[(0, MC)]]
            groups += [[(MC * (2 * g + 1), MC), (MC * (2 * g + 2), MC)]
                       for g in range(7)]
            groups += [[(MSH - 512, 256), (MSH - 256, 128)], [(MSH - 128, 128)]]
            for gi, chunks in enumerate(groups):
                last = gi == len(groups) - 1
                pms = [
                    psp.tile([P, MC], f32, name="psp")
                    for i in range(len(chunks) * NOT)
                ]
                # it-order matches w-copy completion (psum accumulation
                # commutes), so the first m-group never waits on a copy
                it_order = (0, 4, 1, 2, 5, 3, 6, 7) if gi == 0 else range(KT)
                for ot in range(NOT):
                    for idx, it in enumerate(it_order):
                        for ci, (m0, ml) in enumerate(chunks):
                            cc, off = divmod(m0, MC)
                            nc.tensor.matmul(
                                pms[len(chunks) * ot + ci][:, 0:ml],
                                w_sb[:, it, ot * P : (ot + 1) * P],
                                xt_sb[:, cc, it, off : off + ml],
                                start=(idx == 0),
                                stop=(idx == KT - 1),
                            )
                if last:
                    (m0, ml) = chunks[0]
                    yl = ycopy.tile([P, NOT, 128], bf16, name="ylast")
                    nc.vector.tensor_copy(yl[:, 0, :], pms[0][:, 0:ml])
                    nc.scalar.copy(yl[:, 1, :], pms[1][:, 0:ml])
                    nc.sync.dma_start(y_out.ap()[:, :, m0 : m0 + ml], yl[:])
                else:
                    for ot in range(NOT):
                        for ci, (m0, ml) in enumerate(chunks):
                            yt = ycopy.tile([P, MC], bf16, name="yt")
                            nc.vector.tensor_copy(
                                yt[:, 0:ml], pms[len(chunks) * ot + ci][:, 0:ml]
                            )
                            nc.scalar.dma_start(
                                y_out.ap()[:, ot, m0 : m0 + ml], yt[:, 0:ml]
                            )

    nc.compile()
    return nc


def _get_nc():
    if "nc" not in _CACHE:
        _CACHE["nc"] = _build_nc()
    return _CACHE["nc"]


def _make_in_maps(x, A, B, C):
    x2 = np.ascontiguousarray(x, dtype=np.float32).reshape(ROWS, D)
    at = np.asarray(A, np.float32).reshape(D, KT, P).transpose(2, 1, 0)  # [P,KT,D]
    bt = np.asarray(B, np.float32).reshape(KT, P, D).transpose(1, 0, 2)  # [P,KT,D]
    xts = []
    for rg in range(RG):
        shard = x2[rg * MSH : (rg + 1) * MSH]  # [MSH, D]
        xts.append(
            np.ascontiguousarray(
                shard.reshape(NMC, MC, KT, P).transpose(3, 0, 2, 1)
            ).astype(BF16)
        )
    in_maps = []
    for c in range(NCORES):
        rg, cq = divmod(c, CQ)
        csl = np.asarray(C, np.float32)[cq * OD : (cq + 1) * OD, :]  # [OD, D]
        ct = csl.T.reshape(KT, P, OD).transpose(1, 0, 2)  # [P,KT,OD]
        pk = np.empty((P, PW), dtype=np.float32)
        for kt in range(KT):
            pk[:, kt * KTW : kt * KTW + OD] = ct[:, kt, :]
            pk[:, kt * KTW + OD : (kt + 1) * KTW] = at[:, kt, :]
        for jt in range(KT):
            pk[:, BTO + jt * D : BTO + (jt + 1) * D] = bt[:, jt, :]
        in_maps.append({"pk_in": pk.astype(BF16), "xt": xts[rg]})
    return in_maps


def _install_ntff_hook():
    """The agent image's ``antenv`` lacks ``axon_hooks``; recreate it and
    register the ctypes-based NTFF profile hook (same as trn_boot's
    ``_ntff_profile_via_ctypes``) so ``trace=True`` yields exec_time_ns."""
    import contextlib
    import ctypes
    import types

    if "antenv.axon_hooks" in sys.modules:
        return True
    so_path = "/opt/axon/libaxon_pjrt.so"
    if not os.path.exists(so_path):
        return False
    lib = ctypes.CDLL(so_path)
    if not hasattr(lib, "axon_start_nrt_profile"):
        return False
    lib.axon_start_nrt_profile.argtypes = [
        ctypes.POINTER(ctypes.c_int64),
        ctypes.c_size_t,
    ]
    lib.axon_start_nrt_profile.restype = ctypes.c_int64
    lib.axon_stop_nrt_profile.argtypes = [ctypes.c_char_p]
    lib.axon_stop_nrt_profile.restype = ctypes.c_int64

    @contextlib.contextmanager
    def _hook(output_dir, device_ids):
        import jax

        jax.devices()
        if device_ids:
            ids = (ctypes.c_int64 * len(device_ids))(*device_ids)
            rc = lib.axon_start_nrt_profile(ids, len(device_ids))
        else:
            rc = lib.axon_start_nrt_profile(None, 0)
        if rc != 0:
            raise RuntimeError(f"axon_start_nrt_profile rc={rc}")
        try:
            yield
        finally:
            n = lib.axon_stop_nrt_profile(str(output_dir).encode())
            print(f"ntff profile: {n} file(s) written to {output_dir}")

    mod = types.ModuleType("antenv.axon_hooks")
    _state = {"hook": _hook}
    mod.set_axon_ntff_profile_hook = lambda h: _state.__setitem__("hook", h)
    mod.get_axon_ntff_profile_hook = lambda: _state["hook"]
    sys.modules["antenv.axon_hooks"] = mod
    import antenv

    antenv.axon_hooks = mod
    return True


def run(x, A, B, C, trace=False):
    """Run on hardware; returns (y_full, exec_time_ns_or_None)."""
    from concourse import bass_utils
    from concourse.bass_interp import get_hw_module

    if trace and not _install_ntff_hook():
        trace = False
    if trace:
        # upload_artifacts pushes the NEFF dir to a remote bucket; in this
        # sandbox that can fail AFTER a successful run, losing the results.
        # Degrade to the local path. (Only touches the tracing dev path.)
        if not getattr(bass_utils.upload_artifacts, "_safe", False):
            _orig_upload = bass_utils.upload_artifacts

            def _safe_upload(tmpdir):
                try:
                    return _orig_upload(tmpdir)
                except Exception as e:
                    print(f"upload_artifacts skipped ({type(e).__name__}): {e}")
                    return str(tmpdir)

            _safe_upload._safe = True
            bass_utils.upload_artifacts = _safe_upload

    nc = _get_nc()
    in_maps = _make_in_maps(x, A, B, C)

    old_m = nc.m
    nc.m = get_hw_module(nc.m)
    try:
        res = bass_utils.run_bass_kernel_spmd(
            nc, in_maps, core_ids=list(range(NCORES)), trace=trace
        )
    finally:
        nc.m = old_m

    y2 = np.empty((ROWS, D), dtype=np.float32)
    for c in range(NCORES):
        rg, cq = divmod(c, CQ)
        arr = res.results[c]["y_out"]  # [P, NOT, MSH]
        yT = arr.transpose(1, 0, 2).reshape(OD, MSH)
        y2[rg * MSH : (rg + 1) * MSH, cq * OD : (cq + 1) * OD] = (
            yT.T.astype(np.float32)
        )
    return y2.reshape(BATCH, SEQ, D), res.exec_time_ns


def kernel(x, A, B, C):
    y, _ = run(x, A, B, C, trace=False)
    return y


# revision 12
# speedup vs baseline: 1.0159x; 1.0159x over previous
"""Trainium2 Bass kernel for BlockDecomposedSSMAttention.

Math: y[b,s,:] = x[b,s,:] @ B.T @ A @ C.T   (no cross-block recurrence)
 ==>  y = x @ W  with  W = B.T @ A @ C.T

Distribution over the 8 NeuronCores (grid = 2 row-groups x 4 col-quarters):
  core c = (rg, cq):  computes y[rg*8192:(rg+1)*8192, cq*256:(cq+1)*256]
  - x rows are split 2 ways (8192 rows/core, read by 4 cores each).
  - Each core only needs W[:, cq*256:(cq+1)*256], so the W-build stages
    shrink 4x vs full-W-per-core:  T = A @ Ct_q   (1024x256)
                                   W_q = B.T @ T  (1024x256)

Timing model (measured: preamble ~7us, first DMA byte ~8.4us, per-core
input stream capped ~350 GB/s with no cross-queue aggregation,
216ns/512-row matmul; collectives are 5-10x degraded here, so the
duplicated per-core build beats any cross-core W sharing):
  - params are HOST-PACKED into one tensor in exact consumption order
    (per kt: ct 256 cols | at 1024 cols, then bt), so one small first
    piece (0.31 MB, 2.5-KB lines) starts the PE at ~10us and the build
    runs PE-bound to ~24us with zero issue-order stalls.
  - x is chunk-major ([P, chunk, kt, m] -> 8-KB DMA lines) and streams
    JIT behind params under the 55.4us main loop.
  - main loop: W stationary, x moving at N=512, 4 psum banks per
    m-group; y.T out via gpsimd SWDGE (off the input queue). y_out is
    [P, ot, m] so the final 256-row group drains with ONE sync-HWDGE
    DMA right after its two (vector+scalar, parallel) psum copies.
  - All matmul operands bf16 (1 cyc/row), PSUM fp32; y written bf16,
    host transposes + upcasts. Host does layout marshalling only.
"""

import os
import sys

import numpy as np

if "/opt/trn_rl_repo" not in sys.path:
    sys.path.insert(0, "/opt/trn_rl_repo")

import ml_dtypes

BF16 = ml_dtypes.bfloat16

BATCH, SEQ, D = 4, 4096, 1024
NCORES = 8
RG, CQ = 2, 4                 # row-groups x col-quarters
ROWS = BATCH * SEQ            # 16384
MSH = ROWS // RG              # 8192 rows per core
OD = D // CQ                  # 256 output cols per core
P = 128
KT = D // P                   # 8 contraction tiles
MC = 512                      # moving chunk of m in the main loop
NMC = MSH // MC               # 16 m-chunks
NOT = OD // P                 # 2 o'-tiles

KTW = OD + D                  # 1280 packed param cols per kt (ct | at)
BTO = KT * KTW                # 10240: offset of the bt section
PW = BTO + KT * D             # 18432 packed param cols total

_CACHE: dict = {}


def _build_nc():
    import concourse.mybir as mybir
    import concourse.tile as tile
    from concourse import bacc

    f32 = mybir.dt.float32
    bf16 = mybir.dt.bfloat16

    nc = bacc.Bacc(
        "TRN2", target_bir_lowering=False, debug=False, num_devices=NCORES,
        num_swdge_queues=1,
    )

    # Per-core inputs (bf16, contraction dim on partitions):
    #   pk [kp, kt*1280 + o]        = C[cq*256+o, kt*128+kp]   (ct slice)
    #   pk [kp, kt*1280 + 256 + j]  = A[j, kt*128+kp]          (A.T)
    #   pk [jp, 10240 + jt*1024 + i] = B[jt*128+jp, i]         (B)
    #   xt [ip, c, io, m] = x2[rg*8192 + c*512 + m, io*128+ip] (x shard)
    pk_in = nc.dram_tensor("pk_in", [P, PW], bf16, kind="ExternalInput")
    xt = nc.dram_tensor("xt", [P, NMC, KT, MC], bf16, kind="ExternalInput")
    # y.T shard as [p, o', m]; host transposes + upcasts.
    y_out = nc.dram_tensor("y_out", [P, NOT, MSH], bf16, kind="ExternalOutput")

    def ct_ap(sb, kt):
        return sb[:, kt * KTW : kt * KTW + OD]

    def at_ap(sb, kt, jt):
        o = kt * KTW + OD + jt * P
        return sb[:, o : o + P]

    def bt_ap(sb, jt, it):
        o = BTO + jt * D + it * P
        return sb[:, o : o + P]

    with tile.TileContext(nc) as tc:
        with (
            tc.tile_pool(name="big", bufs=1) as big,
            tc.tile_pool(name="ycopy", bufs=8) as ycopy,
            tc.tile_pool(name="psp", bufs=8, space="PSUM") as psp,
        ):
            pk_sb = big.tile([P, PW], bf16)
            t_sb = big.tile([P, KT, OD], bf16)
            w_sb = big.tile([P, KT, OD], bf16)
            xt_sb = big.tile([P, NMC, KT, MC], bf16)

            # ---- input DMAs, all on the single Sync HWDGE queue in exact
            # consumption order (FIFO; per-core bandwidth doesn't aggregate
            # across queues, so one well-ordered stream is optimal). y goes
            # out via the otherwise-idle Scalar HWDGE queue: off the input
            # FIFO, and avoids the 3.4us SWDGE drain in the postamble.
            def pget(lo, hi):
                nc.sync.dma_start(pk_sb[:, lo:hi], pk_in.ap()[:, lo:hi])

            # per-kt pieces: semaphore granularity matches stage-1's
            # 0.87us/kt consumption, so pacing stalls stay sub-100ns.
            # kt0 is split after at-jt3: the first 4 matmuls bridge the
            # second piece's ~0.9us completion-semaphore latency.
            pget(0, OD + 4 * P)
            pget(OD + 4 * P, KTW)
            for kt in range(1, KT):
                pget(kt * KTW, (kt + 1) * KTW)
            # per-jt bt pieces: stage-2's first matmuls no longer wait on a
            # 1-MiB block; consumption granularity 0.87us/jt matches.
            for jt in range(KT):
                pget(BTO + jt * D, BTO + (jt + 1) * D)
            # x stream: per-chunk pieces; with the ci-outer main loop each
            # group only gates on its FIRST chunk (3.46us bridge for the 2nd)
            for c in range(NMC):
                nc.sync.dma_start(xt_sb[:, c : c + 1], xt.ap()[:, c : c + 1])

            # ---- stage 1: T = A @ Ct_q  [1024 x 256], kt-outer over a
            # single 8-bank pass (j-tiles 0..7), paced by the packed stream.
            ps1 = [psp.tile([P, MC], f32, name="psp") for j in range(KT)]
            for kt in range(KT):
                for jt in range(KT):
                    nc.tensor.matmul(
                        ps1[jt][:, 0:OD],
                        at_ap(pk_sb, kt, jt),
                        ct_ap(pk_sb, kt),
                        start=(kt == 0),
                        stop=(kt == KT - 1),
                    )
            for jt in range(KT):
                if jt < 4:
                    nc.vector.tensor_copy(t_sb[:, jt, :], ps1[jt][:, 0:OD])
                else:
                    nc.scalar.copy(t_sb[:, jt, :], ps1[jt][:, 0:OD])

            # ---- stage 2: W_q = B.T @ T  [1024 x 256], jt-outer over a
            # single 8-bank pass (it-tiles 0..7), paced by the bt stream.
            ps2 = [psp.tile([P, MC], f32, name="psp") for i in range(KT)]
            for jt in range(KT):
                for it in range(KT):
                    nc.tensor.matmul(
                        ps2[it][:, 0:OD],
                        bt_ap(pk_sb, jt, it),
                        t_sb[:, jt, :],
                        start=(jt == 0),
                        stop=(jt == KT - 1),
                    )
            # vector (fast) copies it 0-3, scalar (slower ACTIVATE) 4-7;
            # the main loop consumes it-tiles in copy-completion order.
            for it in range(KT):
                if it < 4:
                    nc.vector.tensor_copy(w_sb[:, it, :], ps2[it][:, 0:OD])
                else:
                    nc.scalar.copy(w_sb[:, it, :], ps2[it][:, 0:OD])

            # ---- main: y_q.T = W_q.T @ x.T  [256 x 8192] ----
            # W stationary (reused across m), x moving at N=512.
            # groups: pairs of 512-chunks (= one 2 MiB x DMA piece) -> 4 psum
            # banks per group; final group is a single 256-row chunk drained
            # by ONE dma on the (by then idle) sync HWDGE.
            groups = [[(0, MC)]]
            groups += [[(MC * (2 * g + 1), MC), (MC * (2 * g + 2), MC)]
                       for g in range(7)]
            groups += [[(MSH - 512, 256), (MSH - 256, 128)], [(MSH - 128, 128)]]
            for gi, chunks in enumerate(groups):
                last = gi == len(groups) - 1
                pms = [
                    psp.tile([P, MC], f32, name="psp")
                    for i in range(len(chunks) * NOT)
                ]
                # it-order matches w-copy completion (psum accumulation
                # commutes), so the first m-group never waits on a copy.
                # ci-outer: a group's 2nd chunk isn't touched until the 1st
                # chunk's 16 matmuls (3.46us) are done -> late-x tolerance.
                it_order = (0, 4, 1, 2, 5, 3, 6, 7) if gi == 0 else range(KT)
                for ci, (m0, ml) in enumerate(chunks):
                    cc, off = divmod(m0, MC)
                    for ot in range(NOT):
                        for idx, it in enumerate(it_order):
                            nc.tensor.matmul(
                                pms[len(chunks) * ot + ci][:, 0:ml],
                                w_sb[:, it, ot * P : (ot + 1) * P],
                                xt_sb[:, cc, it, off : off + ml],
                                start=(idx == 0),
                                stop=(idx == KT - 1),
                            )
                if last:
                    (m0, ml) = chunks[0]
                    yl = ycopy.tile([P, NOT, 128], bf16, name="ylast")
                    nc.vector.tensor_copy(yl[:, 0, :], pms[0][:, 0:ml])
                    nc.scalar.copy(yl[:, 1, :], pms[1][:, 0:ml])
                    nc.sync.dma_start(y_out.ap()[:, :, m0 : m0 + ml], yl[:])
                else:
                    for ot in range(NOT):
                        for ci, (m0, ml) in enumerate(chunks):
                            yt = ycopy.tile([P, MC], bf16, name="yt")
                            nc.vector.tensor_copy(
                                yt[:, 0:ml], pms[len(chunks) * ot + ci][:, 0:ml]
                            )
                            nc.scalar.dma_start(
                                y_out.ap()[:, ot, m0 : m0 + ml], yt[:, 0:ml]
                            )

    nc.compile()
    return nc


def _get_nc():
    if "nc" not in _CACHE:
        _CACHE["nc"] = _build_nc()
    return _CACHE["nc"]


def _make_in_maps(x, A, B, C):
    x2 = np.ascontiguousarray(x, dtype=np.float32).reshape(ROWS, D)
    at = np.asarray(A, np.float32).reshape(D, KT, P).transpose(2, 1, 0)  # [P,KT,D]
    bt = np.asarray(B, np.float32).reshape(KT, P, D).transpose(1, 0, 2)  # [P,KT,D]
    xts = []
    for rg in range(RG):
        shard = x2[rg * MSH : (rg + 1) * MSH]  # [MSH, D]
        xts.append(
            np.ascontiguousarray(
                shard.reshape(NMC, MC, KT, P).transpose(3, 0, 2, 1)
            ).astype(BF16)
        )
    in_maps = []
    for c in range(NCORES):
        rg, cq = divmod(c, CQ)
        csl = np.asarray(C, np.float32)[cq * OD : (cq + 1) * OD, :]  # [OD, D]
        ct = csl.T.reshape(KT, P, OD).transpose(1, 0, 2)  # [P,KT,OD]
        pk = np.empty((P, PW), dtype=np.float32)
        for kt in range(KT):
            pk[:, kt * KTW : kt * KTW + OD] = ct[:, kt, :]
            pk[:, kt * KTW + OD : (kt + 1) * KTW] = at[:, kt, :]
        for jt in range(KT):
            pk[:, BTO + jt * D : BTO + (jt + 1) * D] = bt[:, jt, :]
        in_maps.append({"pk_in": pk.astype(BF16), "xt": xts[rg]})
    return in_maps


def _install_ntff_hook():
    """The agent image's ``antenv`` lacks ``axon_hooks``; recreate it and
    register the ctypes-based NTFF profile hook (same as trn_boot's
    ``_ntff_profile_via_ctypes``) so ``trace=True`` yields exec_time_ns."""
    import contextlib
    import ctypes
    import types

    if "antenv.axon_hooks" in sys.modules:
        return True
    so_path = "/opt/axon/libaxon_pjrt.so"
    if not os.path.exists(so_path):
        return False
    lib = ctypes.CDLL(so_path)
    if not hasattr(lib, "axon_start_nrt_profile"):
        return False
    lib.axon_start_nrt_profile.argtypes = [
        ctypes.POINTER(ctypes.c_int64),
        ctypes.c_size_t,
    ]
    lib.axon_start_nrt_profile.restype = ctypes.c_int64
    lib.axon_stop_nrt_profile.argtypes = [ctypes.c_char_p]
    lib.axon_stop_nrt_profile.restype = ctypes.c_int64

    @contextlib.contextmanager
    def _hook(output_dir, device_ids):
        import jax

        jax.devices()
        if device_ids:
            ids = (ctypes.c_int64 * len(device_ids))(*device_ids)
            rc = lib.axon_start_nrt_profile(ids, len(device_ids))
        else:
            rc = lib.axon_start_nrt_profile(None, 0)
        if rc != 0:
            raise RuntimeError(f"axon_start_nrt_profile rc={rc}")
        try:
            yield
        finally:
            n = lib.axon_stop_nrt_profile(str(output_dir).encode())
            print(f"ntff profile: {n} file(s) written to {output_dir}")

    mod = types.ModuleType("antenv.axon_hooks")
    _state = {"hook": _hook}
    mod.set_axon_ntff_profile_hook = lambda h: _state.__setitem__("hook", h)
    mod.get_axon_ntff_profile_hook = lambda: _state["hook"]
    sys.modules["antenv.axon_hooks"] = mod
    import antenv

    antenv.axon_hooks = mod
    return True


def run(x, A, B, C, trace=False):
    """Run on hardware; returns (y_full, exec_time_ns_or_None)."""
    from concourse import bass_utils
    from concourse.bass_interp import get_hw_module

    if trace and not _install_ntff_hook():
        trace = False
    if trace:
        # upload_artifacts pushes the NEFF dir to a remote bucket; in this
        # sandbox that can fail AFTER a successful run, losing the results.
        # Degrade to the local path. (Only touches the tracing dev path.)
        if not getattr(bass_utils.upload_artifacts, "_safe", False):
            _orig_upload = bass_utils.upload_artifacts

            def _safe_upload(tmpdir):
                try:
                    return _orig_upload(tmpdir)
                except Exception as e:
                    print(f"upload_artifacts skipped ({type(e).__name__}): {e}")
                    return str(tmpdir)

            _safe_upload._safe = True
            bass_utils.upload_artifacts = _safe_upload

    nc = _get_nc()
    in_maps = _make_in_maps(x, A, B, C)

    old_m = nc.m
    nc.m = get_hw_module(nc.m)
    try:
        res = bass_utils.run_bass_kernel_spmd(
            nc, in_maps, core_ids=list(range(NCORES)), trace=trace
        )
    finally:
        nc.m = old_m

    y2 = np.empty((ROWS, D), dtype=np.float32)
    for c in range(NCORES):
        rg, cq = divmod(c, CQ)
        arr = res.results[c]["y_out"]  # [P, NOT, MSH]
        yT = arr.transpose(1, 0, 2).reshape(OD, MSH)
        y2[rg * MSH : (rg + 1) * MSH, cq * OD : (cq + 1) * OD] = (
            yT.T.astype(np.float32)
        )
    return y2.reshape(BATCH, SEQ, D), res.exec_time_ns


def kernel(x, A, B, C):
    y, _ = run(x, A, B, C, trace=False)
    return y


# revision 13
# speedup vs baseline: 1.0328x; 1.0166x over previous
"""Trainium2 Bass kernel for BlockDecomposedSSMAttention.

Math: y[b,s,:] = x[b,s,:] @ B.T @ A @ C.T   (no cross-block recurrence)
 ==>  y = x @ W  with  W = B.T @ A @ C.T

Distribution over the 8 NeuronCores (grid = 2 row-groups x 4 col-quarters):
  core c = (rg, cq):  computes y[rg*8192:(rg+1)*8192, cq*256:(cq+1)*256]
  - x rows are split 2 ways (8192 rows/core, read by 4 cores each).
  - Each core only needs W[:, cq*256:(cq+1)*256], so the W-build stages
    shrink 4x vs full-W-per-core:  T = A @ Ct_q   (1024x256)
                                   W_q = B.T @ T  (1024x256)

Timing model (measured: preamble ~7us, first DMA byte ~8.4us, per-core
input stream capped ~350 GB/s with no cross-queue aggregation,
216ns/512-row matmul; collectives are 5-10x degraded here, so the
duplicated per-core build beats any cross-core W sharing):
  - params are HOST-PACKED into one tensor in exact consumption order
    (per kt: ct 256 cols | at 1024 cols, then bt), so one small first
    piece (0.31 MB, 2.5-KB lines) starts the PE at ~10us and the build
    runs PE-bound to ~24us with zero issue-order stalls.
  - x is chunk-major ([P, chunk, kt, m] -> 8-KB DMA lines) and streams
    JIT behind params under the 55.4us main loop.
  - main loop: W stationary, x moving at N=512, 4 psum banks per
    m-group; y.T out via gpsimd SWDGE (off the input queue). y_out is
    [P, ot, m] so the final 256-row group drains with ONE sync-HWDGE
    DMA right after its two (vector+scalar, parallel) psum copies.
  - All matmul operands bf16 (1 cyc/row), PSUM fp32; y written bf16,
    host transposes + upcasts. Host does layout marshalling only.
"""

import os
import sys

import numpy as np

if "/opt/trn_rl_repo" not in sys.path:
    sys.path.insert(0, "/opt/trn_rl_repo")

import ml_dtypes

BF16 = ml_dtypes.bfloat16

BATCH, SEQ, D = 4, 4096, 1024
NCORES = 8
RG, CQ = 2, 4                 # row-groups x col-quarters
ROWS = BATCH * SEQ            # 16384
MSH = ROWS // RG              # 8192 rows per core
OD = D // CQ                  # 256 output cols per core
P = 128
KT = D // P                   # 8 contraction tiles
MC = 512                      # moving chunk of m in the main loop
NMC = MSH // MC               # 16 m-chunks
NOT = OD // P                 # 2 o'-tiles

KTW = OD + D                  # 1280 packed param cols per kt (ct | at)
BTO = KT * KTW                # 10240: offset of the bt section
PW = BTO + KT * D             # 18432 packed param cols total

_CACHE: dict = {}


def _build_nc():
    import concourse.mybir as mybir
    import concourse.tile as tile
    from concourse import bacc

    f32 = mybir.dt.float32
    bf16 = mybir.dt.bfloat16

    nc = bacc.Bacc(
        "TRN2", target_bir_lowering=False, debug=False, num_devices=NCORES,
        num_swdge_queues=1,
    )

    # Per-core inputs (bf16, contraction dim on partitions):
    #   pk [kp, kt*1280 + o]        = C[cq*256+o, kt*128+kp]   (ct slice)
    #   pk [kp, kt*1280 + 256 + j]  = A[j, kt*128+kp]          (A.T)
    #   pk [jp, 10240 + jt*1024 + i] = B[jt*128+jp, i]         (B)
    #   xt [ip, c, io, m] = x2[rg*8192 + c*512 + m, io*128+ip] (x shard)
    pk_in = nc.dram_tensor("pk_in", [P, PW], bf16, kind="ExternalInput")
    xt = nc.dram_tensor("xt", [P, NMC, KT, MC], bf16, kind="ExternalInput")
    # y.T shard as [p, o', m]; host transposes + upcasts.
    y_out = nc.dram_tensor("y_out", [P, NOT, MSH], bf16, kind="ExternalOutput")

    def ct_ap(sb, kt):
        return sb[:, kt * KTW : kt * KTW + OD]

    def at_ap(sb, kt, jt):
        o = kt * KTW + OD + jt * P
        return sb[:, o : o + P]

    def bt_ap(sb, jt, it):
        o = BTO + jt * D + it * P
        return sb[:, o : o + P]

    with tile.TileContext(nc) as tc:
        with (
            tc.tile_pool(name="big", bufs=1) as big,
            tc.tile_pool(name="ycopy", bufs=8) as ycopy,
            tc.tile_pool(name="psp", bufs=8, space="PSUM") as psp,
        ):
            pk_sb = big.tile([P, PW], bf16)
            t_sb = big.tile([P, KT, OD], bf16)
            w_sb = big.tile([P, KT, OD], bf16)
            xt_sb = big.tile([P, NMC, KT, MC], bf16)

            # ---- input DMAs, all on the single Sync HWDGE queue in exact
            # consumption order (FIFO; per-core bandwidth doesn't aggregate
            # across queues, so one well-ordered stream is optimal). y goes
            # out via the otherwise-idle Scalar HWDGE queue: off the input
            # FIFO, and avoids the 3.4us SWDGE drain in the postamble.
            def pget(lo, hi):
                nc.sync.dma_start(pk_sb[:, lo:hi], pk_in.ap()[:, lo:hi])

            # per-kt pieces: semaphore granularity matches stage-1's
            # 0.87us/kt consumption, so pacing stalls stay sub-100ns.
            # kt0 is split after at-jt3: the first 4 matmuls bridge the
            # second piece's ~0.9us completion-semaphore latency.
            pget(0, OD + 4 * P)
            pget(OD + 4 * P, KTW)
            for kt in range(1, KT):
                pget(kt * KTW, (kt + 1) * KTW)
            # per-jt bt pieces: stage-2's first matmuls no longer wait on a
            # 1-MiB block; consumption granularity 0.87us/jt matches.
            for jt in range(KT):
                pget(BTO + jt * D, BTO + (jt + 1) * D)
            # x stream: per-chunk pieces; with the ci-outer main loop each
            # group only gates on its FIRST chunk (3.46us bridge for the 2nd)
            for c in range(NMC):
                nc.sync.dma_start(xt_sb[:, c : c + 1], xt.ap()[:, c : c + 1])

            # ---- stage 1: T = A @ Ct_q  [1024 x 256], kt-outer over a
            # single 8-bank pass (j-tiles 0..7), paced by the packed stream.
            ps1 = [psp.tile([P, MC], f32, name="psp") for j in range(KT)]
            for kt in range(KT):
                for jt in range(KT):
                    nc.tensor.matmul(
                        ps1[jt][:, 0:OD],
                        at_ap(pk_sb, kt, jt),
                        ct_ap(pk_sb, kt),
                        start=(kt == 0),
                        stop=(kt == KT - 1),
                    )
            for jt in range(KT):
                if jt < 4:
                    nc.vector.tensor_copy(t_sb[:, jt, :], ps1[jt][:, 0:OD])
                else:
                    nc.scalar.copy(t_sb[:, jt, :], ps1[jt][:, 0:OD])

            # ---- stage 2: W_q = B.T @ T  [1024 x 256], jt-outer over a
            # single 8-bank pass (it-tiles 0..7), paced by the bt stream.
            ps2 = [psp.tile([P, MC], f32, name="psp") for i in range(KT)]
            for jt in range(KT):
                for it in range(KT):
                    nc.tensor.matmul(
                        ps2[it][:, 0:OD],
                        bt_ap(pk_sb, jt, it),
                        t_sb[:, jt, :],
                        start=(jt == 0),
                        stop=(jt == KT - 1),
                    )
            # vector (fast) copies it 0-3, scalar (slower ACTIVATE) 4-7;
            # the main loop consumes it-tiles in copy-completion order.
            for it in range(KT):
                if it < 4:
                    nc.vector.tensor_copy(w_sb[:, it, :], ps2[it][:, 0:OD])
                else:
                    nc.scalar.copy(w_sb[:, it, :], ps2[it][:, 0:OD])

            # ---- main: y_q.T = W_q.T @ x.T  [256 x 8192] ----
            # W stationary (reused across m), x moving at N=512.
            # groups: pairs of 512-chunks (= one 2 MiB x DMA piece) -> 4 psum
            # banks per group; final group is a single 256-row chunk drained
            # by ONE dma on the (by then idle) sync HWDGE.
            groups = [[(0, MC)]]
            groups += [[(MC * (2 * g + 1), MC), (MC * (2 * g + 2), MC)]
                       for g in range(7)]
            groups += [[(MSH - 512, 256), (MSH - 256, 128)], [(MSH - 128, 128)]]
            for gi, chunks in enumerate(groups):
                last = gi == len(groups) - 1
                pms = [
                    psp.tile([P, MC], f32, name="psp")
                    for i in range(len(chunks) * NOT)
                ]
                # it-order matches w-copy completion (psum accumulation
                # commutes), so the first m-group never waits on a copy.
                # ci-outer: a group's 2nd chunk isn't touched until the 1st
                # chunk's 16 matmuls (3.46us) are done -> late-x tolerance.
                it_order = (0, 4, 1, 2, 5, 3, 6, 7) if gi == 0 else range(KT)
                for ci, (m0, ml) in enumerate(chunks):
                    cc, off = divmod(m0, MC)
                    for ot in range(NOT):
                        for idx, it in enumerate(it_order):
                            nc.tensor.matmul(
                                pms[len(chunks) * ot + ci][:, 0:ml],
                                w_sb[:, it, ot * P : (ot + 1) * P],
                                xt_sb[:, cc, it, off : off + ml],
                                start=(idx == 0),
                                stop=(idx == KT - 1),
                            )
                if last:
                    (m0, ml) = chunks[0]
                    yl = ycopy.tile([P, NOT, 128], bf16, name="ylast")
                    # both on vector: scalar's seq is still issuing the
                    # previous group's y DMAs (~0.6us each) at this point
                    nc.vector.tensor_copy(yl[:, 0, :], pms[0][:, 0:ml])
                    nc.vector.tensor_copy(yl[:, 1, :], pms[1][:, 0:ml])
                    nc.sync.dma_start(y_out.ap()[:, :, m0 : m0 + ml], yl[:])
                else:
                    for ot in range(NOT):
                        for ci, (m0, ml) in enumerate(chunks):
                            yt = ycopy.tile([P, MC], bf16, name="yt")
                            nc.vector.tensor_copy(
                                yt[:, 0:ml], pms[len(chunks) * ot + ci][:, 0:ml]
                            )
                            nc.scalar.dma_start(
                                y_out.ap()[:, ot, m0 : m0 + ml], yt[:, 0:ml]
                            )

    nc.compile()
    return nc


def _get_nc():
    if "nc" not in _CACHE:
        _CACHE["nc"] = _build_nc()
    return _CACHE["nc"]


def _make_in_maps(x, A, B, C):
    x2 = np.ascontiguousarray(x, dtype=np.float32).reshape(ROWS, D)
    at = np.asarray(A, np.float32).reshape(D, KT, P).transpose(2, 1, 0)  # [P,KT,D]
    bt = np.asarray(B, np.float32).reshape(KT, P, D).transpose(1, 0, 2)  # [P,KT,D]
    xts = []
    for rg in range(RG):
        shard = x2[rg * MSH : (rg + 1) * MSH]  # [MSH, D]
        xts.append(
            np.ascontiguousarray(
                shard.reshape(NMC, MC, KT, P).transpose(3, 0, 2, 1)
            ).astype(BF16)
        )
    in_maps = []
    for c in range(NCORES):
        rg, cq = divmod(c, CQ)
        csl = np.asarray(C, np.float32)[cq * OD : (cq + 1) * OD, :]  # [OD, D]
        ct = csl.T.reshape(KT, P, OD).transpose(1, 0, 2)  # [P,KT,OD]
        pk = np.empty((P, PW), dtype=np.float32)
        for kt in range(KT):
            pk[:, kt * KTW : kt * KTW + OD] = ct[:, kt, :]
            pk[:, kt * KTW + OD : (kt + 1) * KTW] = at[:, kt, :]
        for jt in range(KT):
            pk[:, BTO + jt * D : BTO + (jt + 1) * D] = bt[:, jt, :]
        in_maps.append({"pk_in": pk.astype(BF16), "xt": xts[rg]})
    return in_maps


def _install_ntff_hook():
    """The agent image's ``antenv`` lacks ``axon_hooks``; recreate it and
    register the ctypes-based NTFF profile hook (same as trn_boot's
    ``_ntff_profile_via_ctypes``) so ``trace=True`` yields exec_time_ns."""
    import contextlib
    import ctypes
    import types

    if "antenv.axon_hooks" in sys.modules:
        return True
    so_path = "/opt/axon/libaxon_pjrt.so"
    if not os.path.exists(so_path):
        return False
    lib = ctypes.CDLL(so_path)
    if not hasattr(lib, "axon_start_nrt_profile"):
        return False
    lib.axon_start_nrt_profile.argtypes = [
        ctypes.POINTER(ctypes.c_int64),
        ctypes.c_size_t,
    ]
    lib.axon_start_nrt_profile.restype = ctypes.c_int64
    lib.axon_stop_nrt_profile.argtypes = [ctypes.c_char_p]
    lib.axon_stop_nrt_profile.restype = ctypes.c_int64

    @contextlib.contextmanager
    def _hook(output_dir, device_ids):
        import jax

        jax.devices()
        if device_ids:
            ids = (ctypes.c_int64 * len(device_ids))(*device_ids)
            rc = lib.axon_start_nrt_profile(ids, len(device_ids))
        else:
            rc = lib.axon_start_nrt_profile(None, 0)
        if rc != 0:
            raise RuntimeError(f"axon_start_nrt_profile rc={rc}")
        try:
            yield
        finally:
            n = lib.axon_stop_nrt_profile(str(output_dir).encode())
            print(f"ntff profile: {n} file(s) written to {output_dir}")

    mod = types.ModuleType("antenv.axon_hooks")
    _state = {"hook": _hook}
    mod.set_axon_ntff_profile_hook = lambda h: _state.__setitem__("hook", h)
    mod.get_axon_ntff_profile_hook = lambda: _state["hook"]
    sys.modules["antenv.axon_hooks"] = mod
    import antenv

    antenv.axon_hooks = mod
    return True


def run(x, A, B, C, trace=False):
    """Run on hardware; returns (y_full, exec_time_ns_or_None)."""
    from concourse import bass_utils
    from concourse.bass_interp import get_hw_module

    if trace and not _install_ntff_hook():
        trace = False
    if trace:
        # upload_artifacts pushes the NEFF dir to a remote bucket; in this
        # sandbox that can fail AFTER a successful run, losing the results.
        # Degrade to the local path. (Only touches the tracing dev path.)
        if not getattr(bass_utils.upload_artifacts, "_safe", False):
            _orig_upload = bass_utils.upload_artifacts

            def _safe_upload(tmpdir):
                try:
                    return _orig_upload(tmpdir)
                except Exception as e:
                    print(f"upload_artifacts skipped ({type(e).__name__}): {e}")
                    return str(tmpdir)

            _safe_upload._safe = True
            bass_utils.upload_artifacts = _safe_upload

    nc = _get_nc()
    in_maps = _make_in_maps(x, A, B, C)

    old_m = nc.m
    nc.m = get_hw_module(nc.m)
    try:
        res = bass_utils.run_bass_kernel_spmd(
            nc, in_maps, core_ids=list(range(NCORES)), trace=trace
        )
    finally:
        nc.m = old_m

    y2 = np.empty((ROWS, D), dtype=np.float32)
    for c in range(NCORES):
        rg, cq = divmod(c, CQ)
        arr = res.results[c]["y_out"]  # [P, NOT, MSH]
        yT = arr.transpose(1, 0, 2).reshape(OD, MSH)
        y2[rg * MSH : (rg + 1) * MSH, cq * OD : (cq + 1) * OD] = (
            yT.T.astype(np.float32)
        )
    return y2.reshape(BATCH, SEQ, D), res.exec_time_ns


def kernel(x, A, B, C):
    y, _ = run(x, A, B, C, trace=False)
    return y
